# revision 1
# baseline (speedup 1.0000x reference)
"""Causal multi-head attention (B=4, N=2048, C=1024, H=16) on 8 Trainium2 cores.

Sharding: data-parallel over batch (4) x tensor-parallel over heads (2 groups
of 8).  Core c handles batch c//2, head-group c%2.  Each core computes its
heads' attention and a partial output projection; the host sums the two
head-group partials per batch and adds the bias.

Device layout notes (per core):
  - All matmul operands are bf16; accumulation fp32 in PSUM.
  - x, weights are shipped pre-transposed so QKV lands as q^T/k^T [d, n].
  - Scores are computed transposed (S^T[kv, q]) so softmax's exp feeds the
    PV matmul directly without transposing the probability matrix.
  - No max-subtraction in softmax: scores are O(1) (std ~1) by construction,
    exp never overflows fp32.  The causal mask is added via an
    identity-matmul of an additive mask tile into PSUM before the score
    matmul accumulates on top.
  - The softmax denominator comes for free from a 65th all-ones column
    appended to V (row 64 of the PV PSUM output).
  - Output projection consumes attn^T and produces out^T; the host
    transposes while unsharding.
"""

import numpy as np
import ml_dtypes

BF16 = ml_dtypes.bfloat16

B, N, C, H, D = 4, 2048, 1024, 16, 64
HPC = 8            # heads per core
GD = HPC * D       # 512 channels per head-group
P = 128
KC = C // P        # 8 contraction chunks for the projections
SPAN = 512         # query-column span processed per attention step
NSPAN = N // SPAN
NEG = -28672.0     # additive mask; exactly representable in bf16

_CACHE = {}


def _emit_once(tc, mybir, xT_d, wqkT_d, wvT_d, wpT_d, bm_d, id_d, out_d,
               phases):
    nc = tc.nc
    dt = mybir.dt
    f32, bf = dt.float32, dt.bfloat16
    Exp = mybir.ActivationFunctionType.Exp
    MUL = mybir.AluOpType.mult
    do_qkv = "qkv" in phases
    do_attn = "attn" in phases
    do_proj = "proj" in phases

    with (
        tc.tile_pool(name="weights", bufs=1) as wp,
        tc.tile_pool(name="acts", bufs=1) as ab,
        tc.tile_pool(name="small", bufs=4) as sp,
        tc.tile_pool(name="ps", bufs=1, space="PSUM") as ps,
        tc.tile_pool(name="aTp", bufs=2) as aTp,
        tc.tile_pool(name="exp", bufs=4) as exp_pool,
    ):
        # ---------------- input loads (chunked: DMA parallelism + fine deps)
        xk = [[wp.tile([P, N // 2], bf, tag=f"xk{k}_{h2}", name=f"xk{k}_{h2}")
               for h2 in range(2)] for k in range(KC)]
        wqk = [wp.tile([P, 2 * GD], bf, tag=f"wqk{k}", name=f"wqk{k}")
               for k in range(KC)]
        wv = [wp.tile([P, GD], bf, tag=f"wv{k}", name=f"wv{k}")
              for k in range(KC)]
        for k in range(KC):
            for h2 in range(2):
                nc.sync.dma_start(
                    xk[k][h2],
                    xT_d[k * P:(k + 1) * P,
                         h2 * (N // 2):(h2 + 1) * (N // 2)])
            nc.sync.dma_start(wqk[k], wqkT_d[k * P:(k + 1) * P, :])
            nc.sync.dma_start(wv[k], wvT_d[k * P:(k + 1) * P, :])
        wpk = [wp.tile([P, C], bf, tag=f"wpk{k}", name=f"wpk{k}")
               for k in range(GD // P)]
        for k in range(GD // P):
            nc.sync.dma_start(wpk[k], wpT_d[k * P:(k + 1) * P, :])
        bm = wp.tile([P, 2 * SPAN], bf, tag="bm")
        nc.sync.dma_start(bm, bm_d)
        i128 = wp.tile([P, P], bf, tag="i128")
        nc.sync.dma_start(i128, id_d)

        # q^T/k^T rows: per (128-row chunk, 512-col quarter) tiles so
        # attention can start before a chunk's later columns are computed
        qkm = [[ab.tile([P, SPAN], bf, tag=f"qkm{m}_{q}", name=f"qkm{m}_{q}")
                for q in range(4)] for m in range(2 * GD // P)]
        # V per kv-block with an all-ones 65th column per head
        vab = [ab.tile([P, HPC * (D + 1)], bf, tag=f"vab{m}", name=f"vab{m}")
               for m in range(N // P)]

        # PSUM bank budget (8 banks of [128, 512]f32):
        #   qk (QKV groups)     [128, 2, 512] x1  = 2
        #   duo/pp              [128, 2, 512] x2  = 4
        #   oA, oB              [65, 512]     x1  = 2
        def qk_chunk(m):
            if not do_qkv:
                return
            for q in range(4):
                pg = ps.tile([P, SPAN], f32, tag="qk", name=f"pg{m}{q}",
                             bufs=2)
                for k in range(KC):
                    nc.tensor.matmul(
                        pg,
                        wqk[k][:, m * P:(m + 1) * P],
                        xk[k][q // 2][:, (q % 2) * SPAN:(q % 2 + 1) * SPAN],
                        start=(k == 0),
                        stop=(k == KC - 1),
                    )
                nc.vector.tensor_copy(out=qkm[m][q], in_=pg)

        def v_chunk(m4):
            if not do_qkv:
                return
            for sub in range(4):
                m16 = m4 * 4 + sub
                pv = ps.tile([P, SPAN], f32, tag="qk", name=f"pv{m16}",
                             bufs=2)
                nc.vector.memset(vab[m16], 1.0)
                for k in range(KC):
                    nc.tensor.matmul(
                        pv,
                        xk[k][m16 // 8][:, (m16 % 8) * P:(m16 % 8 + 1) * P],
                        wv[k],
                        start=(k == 0),
                        stop=(k == KC - 1),
                    )
                nc.vector.tensor_copy(
                    out=vab[m16].rearrange(
                        "p (h e) -> p h e", h=HPC)[:, :, :D],
                    in_=pv.rearrange("p (h e) -> p h e", h=HPC),
                )

        def attn_pair(J, hp, acT):
            if not do_attn:
                return
            nblk = 4 * (J + 1)
            qs = J * SPAN
            outs = (
                ps.tile([65, SPAN], f32, tag="oA", name="oA", bufs=1),
                ps.tile([65, SPAN], f32, tag="oB", name="oB", bufs=1),
            )
            def emit_pv(ex, j2, lo):
                for hi in (0, 1):
                    h = 2 * hp + hi
                    nc.tensor.matmul(
                        outs[hi][:, lo:],
                        vab[j2][:, h * (D + 1):(h + 1) * (D + 1)],
                        ex[:, hi, lo:],
                        start=(j2 == 0),
                        stop=(j2 == nblk - 1),
                    )

            pend = None  # software pipeline: PV one block behind scores/exp
            for j2 in range(nblk):
                duo = ps.tile([P, 2, SPAN], f32, tag="duo", bufs=2)
                dtg = j2 - 4 * J   # >=0: diagonal block index
                lo = P * dtg if dtg >= 0 else 0  # first live column
                diag = dtg >= 0
                if diag:
                    # triangle masks for both heads first, so the two score
                    # matmuls issue back-to-back and row-pack concurrently
                    for hi in (0, 1):
                        nc.tensor.matmul(
                            duo[:, hi, lo:lo + P], i128,
                            bm[:, SPAN:SPAN + P],
                            start=True, stop=False,
                        )
                for hi in (0, 1):
                    nc.tensor.matmul(
                        duo[:, hi, lo:],
                        qkm[4 + hp][j2 // 4][64 * hi:64 * (hi + 1),
                                             (j2 % 4) * P:(j2 % 4 + 1) * P],
                        qkm[hp][J][64 * hi:64 * (hi + 1), lo:],
                        start=not diag,
                        stop=True,
                    )
                ex = exp_pool.tile([P, 2, SPAN], bf, tag="ex")
                nc.scalar.activation(ex[:, :, lo:], duo[:, :, lo:], Exp)
                if pend is not None:
                    emit_pv(*pend)
                pend = (ex, j2, lo)
            emit_pv(*pend)
            for hi in (0, 1):
                o = outs[hi]
                rc = sp.tile([1, SPAN], f32, tag="rc")
                nc.vector.reciprocal(rc, o[64:65, :])
                bc = sp.tile([64, SPAN], f32, tag="bc")
                nc.gpsimd.partition_broadcast(bc, rc)
                nc.vector.tensor_tensor(
                    acT[64 * hi:64 * (hi + 1), hp, :], o[0:64, :], bc, MUL,
                )

        def proj_span(J, acT):
            if not do_proj:
                return
            qs = J * SPAN
            for mo in range(C // P):
                pp = ps.tile([P, SPAN], f32, tag="duo", name=f"pp{mo}",
                             bufs=2)
                for k in range(GD // P):
                    nc.tensor.matmul(
                        pp,
                        wpk[k][:, mo * P:(mo + 1) * P],
                        acT[:, k, :],
                        start=(k == 0),
                        stop=(k == GD // P - 1),
                    )
                ob = sp.tile([P, SPAN], f32, tag="ob")
                nc.vector.tensor_copy(out=ob, in_=pp)
                nc.sync.dma_start(out_d[mo * P:(mo + 1) * P, qs:qs + SPAN],
                                  ob)

        # Interleaved emission: attention (span J, pair hp) needs qkm[hp],
        # qkm[4+hp], vab[0..4J+3]; unblock hp pairs of span 0 early so ACT
        # overlaps the QKV phase.
        acTs = [aTp.tile([P, GD // P, SPAN], bf, tag="acT", name=f"acT{J}")
                for J in range(NSPAN)]
        qk_chunk(0)
        qk_chunk(4)
        v_chunk(0)
        attn_pair(0, 0, acTs[0])
        qk_chunk(1)
        qk_chunk(5)
        attn_pair(0, 1, acTs[0])
        qk_chunk(2)
        qk_chunk(6)
        attn_pair(0, 2, acTs[0])
        qk_chunk(3)
        qk_chunk(7)
        attn_pair(0, 3, acTs[0])
        v_chunk(1)
        attn_pair(1, 0, acTs[1])
        proj_span(0, acTs[0])
        for hp in range(1, 4):
            attn_pair(1, hp, acTs[1])
        v_chunk(2)
        attn_pair(2, 0, acTs[2])
        proj_span(1, acTs[1])
        for hp in range(1, 4):
            attn_pair(2, hp, acTs[2])
        v_chunk(3)
        attn_pair(3, 0, acTs[3])
        proj_span(2, acTs[2])
        for hp in range(1, 4):
            attn_pair(3, hp, acTs[3])
        proj_span(3, acTs[3])


def _emit(tc, mybir, reps=1, phases=("qkv", "attn", "proj")):
    nc = tc.nc
    dt = mybir.dt
    f32, bf = dt.float32, dt.bfloat16

    xT_d = nc.dram_tensor("xT", [C, N], bf, kind="ExternalInput").ap()
    wqkT_d = nc.dram_tensor("wqkT", [C, 2 * GD], bf, kind="ExternalInput").ap()
    wvT_d = nc.dram_tensor("wvT", [C, GD], bf, kind="ExternalInput").ap()
    wpT_d = nc.dram_tensor("wpT", [GD, C], bf, kind="ExternalInput").ap()
    bm_d = nc.dram_tensor("BM", [P, 2 * SPAN], bf, kind="ExternalInput").ap()
    id_d = nc.dram_tensor("I128", [P, P], bf, kind="ExternalInput").ap()
    out_d = nc.dram_tensor("outT", [C, N], f32, kind="ExternalOutput").ap()

    for _rep in range(reps):
        _emit_once(tc, mybir, xT_d, wqkT_d, wvT_d, wpT_d, bm_d, id_d, out_d,
                   phases)


def _get_module(reps=1, phases=("qkv", "attn", "proj")):
    key = (reps, tuple(phases))
    if key not in _CACHE:
        import concourse.tile as tile
        from concourse import bacc, mybir

        nc = bacc.Bacc("TRN2", target_bir_lowering=False, debug=False,
                       num_devices=8)
        with tile.TileContext(nc) as tc:
            _emit(tc, mybir, reps=reps, phases=phases)
        nc.compile()
        _CACHE[key] = nc
    return _CACHE[key]


def _host_inputs(x, w_qkv, w_proj):
    scale = D ** -0.5
    bmask = np.full((P, 2 * SPAN), NEG, np.float32)
    for p in range(P):
        bmask[p, p + SPAN:] = 0.0
    bmask = bmask.astype(BF16)
    ident = np.eye(P, dtype=BF16)
    in_maps = []
    for core in range(8):
        b, g = core // 2, core % 2
        rows = slice(g * GD, (g + 1) * GD)
        wq = w_qkv[0 * C:1 * C][rows] * scale
        wk = w_qkv[1 * C:2 * C][rows]
        wv = w_qkv[2 * C:3 * C][rows]
        in_maps.append({
            "xT": np.ascontiguousarray(x[b].T).astype(BF16),
            "wqkT": np.ascontiguousarray(
                np.concatenate([wq, wk], axis=0).T).astype(BF16),
            "wvT": np.ascontiguousarray(wv.T).astype(BF16),
            "wpT": np.ascontiguousarray(w_proj[:, rows].T).astype(BF16),
            "BM": bmask,
            "I128": ident,
        })
    return in_maps


def kernel(x, w_qkv, w_proj, b_proj, _trace=False):
    from concourse.bass_utils import run_bass_kernel_spmd

    nc = _get_module()
    in_maps = _host_inputs(np.asarray(x, np.float32),
                           np.asarray(w_qkv, np.float32),
                           np.asarray(w_proj, np.float32))
    res = run_bass_kernel_spmd(nc, in_maps, core_ids=list(range(8)),
                               trace=_trace)
    outs = [r["outT"] for r in res.results]
    out = np.empty((B, N, C), np.float32)
    bp = np.asarray(b_proj, np.float32)[None, :]
    for b in range(B):
        out[b] = outs[2 * b].T + outs[2 * b + 1].T + bp
    if _trace:
        kernel._last_results = res
    return out



# revision 31
# speedup vs baseline: 1.3130x; 1.3130x over previous
"""Causal multi-head attention (B=4, N=2048, C=1024, H=16) on 8 Trainium2 cores.

Sharding: data-parallel over batch (4) x tensor-parallel over heads (2 groups
of 8).  Core c handles batch c//2, head-group c%2.  Each core computes its
heads' attention and a partial output projection; the host sums the two
head-group partials per batch and adds the bias.

Device layout notes (per core):
  - All matmul operands are bf16; accumulation fp32 in PSUM.
  - x, weights are shipped pre-transposed so QKV lands as q^T/k^T [d, n].
  - Scores are computed transposed (S^T[kv, q]) so the exp'd probabilities
    feed the PV matmul directly as the stationary operand.
  - No max-subtraction in softmax: scores are O(1) (std ~1) by construction,
    exp never overflows.  The causal mask is applied multiplicatively on the
    exp'd probabilities (DVE), so the tensor engine never touches it.
  - PV is oriented out[q, d]: stationary = ex block [kv, q-chunk], moving =
    V [kv, d] with a 65th all-ones column that yields the softmax denominator
    per q as an extra output column.  The division is then a per-partition
    scalar multiply, and a small PE transpose restores act^T [d, q] for the
    output projection.
  - The per-block attention work (scores+PV) is cheaper on the tensor engine
    than exp is on the activation engine, and the PE queue issues strictly
    in order, so independent GEMM work (qkv quarters 1-3, V chunks, the
    output projection) is interleaved between attention blocks at 1-2 matmul
    granularity to keep the tensor engine from stalling on exp.
"""

from collections import deque

import numpy as np
import ml_dtypes

BF16 = ml_dtypes.bfloat16

B, N, C, H, D = 4, 2048, 1024, 16, 64
HPC = 8            # heads per core
GD = HPC * D       # 512 channels per head-group
P = 128
KC = C // P        # 8 contraction chunks for the projections
SPAN = 512         # query-column span per attention step
NSPAN = N // SPAN

_CACHE = {}


def _emit_once(tc, mybir, xT_d, wqkT_d, wvT_d, wpT_d, tri_d, id_d, out_d):
    nc = tc.nc
    dt = mybir.dt
    f32, bf = dt.float32, dt.bfloat16
    Exp = mybir.ActivationFunctionType.Exp
    MUL = mybir.AluOpType.mult

    with (
        tc.tile_pool(name="weights", bufs=1) as wp,
        tc.tile_pool(name="acts", bufs=1) as ab,
        tc.tile_pool(name="aq", bufs=4) as aqp,
        tc.tile_pool(name="rc", bufs=4) as rcp,
        tc.tile_pool(name="ob", bufs=3) as obp,
        tc.tile_pool(name="acT", bufs=4) as aTp,
        tc.tile_pool(name="exp", bufs=4) as exp_pool,
        tc.tile_pool(name="duo", bufs=2, space="PSUM") as duop,
        tc.tile_pool(name="pv", bufs=1, space="PSUM") as pvp,
        tc.tile_pool(name="wk", bufs=2, space="PSUM") as wkp,
    ):
        # ---------------- inputs.  Order = arrival order: interleave the
        # w_qk chunks with the x halves the quarter-0 GEMM needs first.
        xk = [wp.tile([P, N], bf, tag=f"xk{k}", name=f"xk{k}")
              for k in range(KC)]
        wqkm = wp.tile([P, KC, 2 * GD], bf, tag="wqkm")
        wvm = wp.tile([P, KC, GD], bf, tag="wvm")
        wpm = wp.tile([P, GD // P, C], bf, tag="wpm")
        tri = wp.tile([P, 2, P], bf, tag="tri")
        i128 = wp.tile([P, P], bf, tag="i128")

        # Small/late weight loads go through the Pool SWDGE queue so they
        # don't wait behind the 16 big input DMAs on HWDGE.
        wqk_src = wqkT_d.rearrange("(a p) o -> p a o", p=P)
        HN = N // 2
        nc.gpsimd.dma_start(tri, tri_d)
        nc.gpsimd.dma_start(i128, id_d)
        for k in range(KC):
            nc.sync.dma_start(wqkm[:, k, :], wqk_src[:, k, :])
            nc.sync.dma_start(xk[k][:, 0:HN], xT_d[k * P:(k + 1) * P, 0:HN])
        wv_src = wvT_d.rearrange("(a p) o -> p a o", p=P)
        nc.gpsimd.dma_start(wvm[:, 0:4, :], wv_src[:, 0:4, :])
        nc.gpsimd.dma_start(wvm[:, 4:8, :], wv_src[:, 4:8, :])
        nc.gpsimd.dma_start(wpm, wpT_d.rearrange("(a p) o -> p a o", p=P))
        for k in range(KC):
            nc.sync.dma_start(xk[k][:, HN:N], xT_d[k * P:(k + 1) * P, HN:N])

        # q^T/k^T tiles per (128-row chunk m, 512-token quarter)
        # m 0..3 = q rows of pairs 0..3, m 4..7 = k rows of pairs 0..3.
        qkm = [[ab.tile([P, SPAN], bf, tag=f"qkm{m}_{q}", name=f"qkm{m}_{q}")
                for q in range(4)] for m in range(2 * GD // P)]
        # V per kv-block: 8 heads x (64 dims + all-ones 65th column)
        vab = [ab.tile([P, HPC * (D + 1)], bf, tag=f"vab{m}", name=f"vab{m}")
               for m in range(N // P)]
        acTs = []

        # ---------------- filler thunks: (pe_cost, closure) pairs, each
        # emitting one instruction; the attention loop pulls them between
        # blocks to keep the PE queue fed.
        MMC = 213.0  # pe cost of one 512-col filler matmul

        def qk_thunks(m, qq):
            st = {}
            out = []

            def mm(k):
                if k == 0:
                    st["pg"] = wkp.tile([P, SPAN], f32, tag="wk",
                                        name=f"pg{m}{qq}")
                nc.tensor.matmul(
                    st["pg"], wqkm[:, k, m * P:(m + 1) * P],
                    xk[k][:, qq * SPAN:(qq + 1) * SPAN],
                    start=(k == 0), stop=(k == KC - 1),
                )
            for k in range(KC):
                out.append((MMC, lambda k=k: mm(k)))
            out.append((0.0, lambda: nc.vector.tensor_copy(
                out=qkm[m][qq], in_=st["pg"])))
            return out

        def v_thunks(m16):
            st = {}
            out = [(0.0, lambda: nc.gpsimd.memset(vab[m16], 1.0))]

            def mm(k):
                if k == 0:
                    st["pv"] = wkp.tile([P, SPAN], f32, tag="wk",
                                        name=f"pvw{m16}")
                nc.tensor.matmul(
                    st["pv"], xk[k][:, m16 * P:(m16 + 1) * P], wvm[:, k, :],
                    start=(k == 0), stop=(k == KC - 1),
                )
            for k in range(KC):
                out.append((MMC, lambda k=k: mm(k)))
            out.append((0.0, lambda: nc.vector.tensor_copy(
                out=vab[m16].rearrange("p (h e) -> p h e", h=HPC)[:, :, :D],
                in_=st["pv"].rearrange("p (h e) -> p h e", h=HPC))))
            return out

        def proj_thunks(J, mo):
            st = {}
            out = []

            def mm(k):
                if k == 0:
                    st["pp"] = wkp.tile([P, SPAN], f32, tag="wk",
                                        name=f"pp{J}{mo}")
                nc.tensor.matmul(
                    st["pp"], wpm[:, k, mo * P:(mo + 1) * P],
                    acTs[J][:, k, :],
                    start=(k == 0), stop=(k == GD // P - 1),
                )
            for k in range(GD // P):
                out.append((MMC, lambda k=k: mm(k)))

            def fin():
                ob = obp.tile([P, SPAN], bf, tag="ob")
                nc.vector.tensor_copy(out=ob, in_=st["pp"])
                nc.sync.dma_start(
                    out_d[mo * P:(mo + 1) * P, J * SPAN:(J + 1) * SPAN], ob)
            out.append((0.0, fin))
            return out

        QOFF = (0, 130, 260, 512)  # per-q-chunk offsets in the pv psum tile
        TOFF = (642, 772)          # transpose scratch slots in the pv tile

        # Deferred finalize pipeline: the DVE divide chain (reciprocal + two
        # tensor_scalar) is emitted right after a q-chunk's last PV block,
        # but the dependent PE transpose is deferred ~2 block slots so the
        # in-order PE queue never stalls on the DVE latency.
        slot = [0]          # global block-slot counter
        tdq = deque()       # (ready_slot, transpose thunk)

        def drain_transposes(all_=False, grace=0):
            while tdq and (all_ or tdq[0][0] <= slot[0] + grace):
                tdq.popleft()[1]()

        def attn_pair(J, hp, acc, pull):
            nblk = 4 * (J + 1)
            pvt = pvp.tile([P, 1024], f32, tag="pv", name=f"pv{J}{hp}")

            def finalize(qc):
                q0 = QOFF[qc]
                rc = rcp.tile([P, 2], f32, tag="rc")
                nc.vector.reciprocal(rc, pvt[:, q0 + 64:q0 + 130:65])
                aq = aqp.tile([P, 2, D], bf, tag="aq")
                for hi in (0, 1):
                    nc.vector.tensor_scalar(
                        aq[:, hi, :], pvt[:, q0 + 65 * hi:q0 + 65 * hi + 64],
                        rc[:, hi:hi + 1], None, MUL,
                    )

                def tp():
                    tpt = wkp.tile([P, P], bf, tag="wk", name="tp")
                    nc.tensor.transpose(
                        tpt, aq.rearrange("p h d -> p (h d)"), i128)
                    nc.vector.tensor_copy(
                        out=acc[:, hp, qc * P:(qc + 1) * P], in_=tpt)
                tdq.append((slot[0] + 4, tp))

            def emit_pv(ex, j2, fin=None):
                dtg = j2 - 4 * J
                for qc in range(max(dtg, 0), 4):
                    for hi in (0, 1):
                        h = 2 * hp + hi
                        o = QOFF[qc] + 65 * hi
                        # start=True clears has_written for the whole
                        # PSUM bank on hw, so only the first matmul per bank
                        # (qc 0 / qc 3, head 0) may carry it; sibling chunks'
                        # first writes overwrite via cleared has_written.
                        nc.tensor.matmul(
                            pvt[:, o:o + 65],
                            ex[:, hi, qc * P:(qc + 1) * P],
                            vab[j2][:, h * (D + 1):(h + 1) * (D + 1)],
                            start=(j2 == 0 and hi == 0 and qc in (0, 3)),
                            stop=(dtg == qc),
                        )
                if dtg >= 0:
                    if fin is None:
                        finalize(dtg)
                    else:
                        fin.append(dtg)

            pend = []  # software pipeline: PV two blocks behind scores/exp
            for j2 in range(nblk):
                dtg = j2 - 4 * J
                lo = P * dtg if dtg >= 0 else 0
                duo = duop.tile([P, 2, SPAN], f32, tag="duo")
                for hi in (0, 1):
                    nc.tensor.matmul(
                        duo[:, hi, lo:],
                        qkm[4 + hp][j2 // 4][64 * hi:64 * (hi + 1),
                                             (j2 % 4) * P:(j2 % 4 + 1) * P],
                        qkm[hp][J][64 * hi:64 * (hi + 1), lo:],
                        start=True, stop=True,
                    )
                ex = exp_pool.tile([P, 2, SPAN], bf, tag="ex")
                nc.scalar.activation(ex[:, :, lo:], duo[:, :, lo:], Exp)
                if dtg >= 0:
                    nc.vector.tensor_tensor(
                        ex[:, :, lo:lo + P], ex[:, :, lo:lo + P], tri, MUL)
                pe_blk = 2 * (SPAN - lo) * 0.417
                if len(pend) == 2:
                    ep = pend.pop(0)
                    pe_blk += 27.0 * 2 * (4 - max(ep[1] - 4 * J, 0))
                    emit_pv(*ep)
                pend.append((ex, j2))
                slot[0] += 1
                drain_transposes()
                pull(2 * (SPAN - lo) * 0.85 + 190.0 - pe_blk)
            fin = []
            for p_ in pend:
                emit_pv(*p_, fin=fin)
            for qc in fin:
                finalize(qc)

        # ---------------- prologue: q/k quarter 0, k-outer over 8 psum
        # groups so the PE ramps while x is still streaming in.
        duoA = duop.tile([P, 2, SPAN], f32, tag="duo", name="duoA")
        duoB = duop.tile([P, 2, SPAN], f32, tag="duo", name="duoB")
        wkA = wkp.tile([P, SPAN], f32, tag="wk", name="wkA")
        wkB = wkp.tile([P, SPAN], f32, tag="wk", name="wkB")
        pv0 = pvp.tile([P, 1024], f32, tag="pv", name="pvpro")
        groups = [duoA[:, 0, :], duoA[:, 1, :], duoB[:, 0, :], duoB[:, 1, :],
                  wkA, wkB, pv0[:, 0:SPAN], pv0[:, SPAN:]]
        for k in range(KC):
            for m in range(8):
                nc.tensor.matmul(
                    groups[m], wqkm[:, k, m * P:(m + 1) * P],
                    xk[k][:, 0:SPAN],
                    start=(k == 0), stop=(k == KC - 1),
                )
        for m in range(8):
            nc.vector.tensor_copy(out=qkm[m][0], in_=groups[m])
        # ---------------- spans: attention with adaptively interleaved
        # filler.  Each filler stage has an availability slot (when its
        # inputs exist) and a deadline slot (when its outputs are consumed).
        # Deadline-forced pulls spread mandatory work evenly; opportunistic
        # pulls fire whenever the running ACT-minus-PE balance says the
        # tensor engine is about to starve behind exp.
        def qk_pair(m, qq):
            return qk_thunks(m, qq) + qk_thunks(4 + m, qq)

        def v_chunk_thunks(m4):
            return [t for s in range(4) for t in v_thunks(4 * m4 + s)]

        # Pair processing order: interleave the ACT-heavy span-3 pairs with
        # cheap span-1 pairs (the front half has ACT slack), run span 2 in
        # the back with the deferred projections as filler, and end on the
        # cheapest span-0 pairs so the final ACT-bound stretch is short.
        PAIR_ORDER = [(J, hp) for J in range(4) for hp in range(4)]
        starts = {}
        t_ = 0
        for Jh in PAIR_ORDER:
            starts[Jh] = t_
            t_ += 4 * (Jh[0] + 1)
        span_end = {J: max(starts[(J2, h)] + 4 * (J2 + 1)
                           for (J2, h) in PAIR_ORDER if J2 == J)
                    for J in range(4)}
        END = t_
        sched = []  # (avail, deadline, deque of (pe_cost, thunk))

        def add_stage(avail, deadline, items):
            sched.append((avail, deadline, deque(items)))

        for qq in (1, 2, 3):
            for m in range(4):
                dl_q = starts[(qq, m)]
                dl_k = min(starts[(J, m)] + 4 * qq
                           for (J, h) in PAIR_ORDER if J >= qq and h == m)
                add_stage(0, min(dl_q, dl_k), qk_pair(m, qq))
        for c in (1, 2, 3):
            dl_v = min(starts[(J, h)] + 4 * c
                       for (J, h) in PAIR_ORDER if J >= c)
            add_stage(0, dl_v, v_chunk_thunks(c))
        for J in (0, 1, 2):
            add_stage(span_end[J] + 2, END - 2,
                      [t for mo in range(C // P) for t in proj_thunks(J, mo)])
        sched.sort(key=lambda st: st[1])

        def pull(need):
            # Strictly earliest-deadline-first, one stage at a time, so at
            # most one filler accumulation group is open on the wk pool.
            # Pull while this block's ACT-minus-PE deficit is uncovered OR
            # any available stage is behind its deadline pace.
            s = slot[0]
            while True:
                cand = None
                pressured = False
                for st in sched:
                    if st[2] and st[0] <= s:
                        if cand is None:
                            cand = st
                        if len(st[2]) > max(st[1] - s - 1, 0) * 3:
                            pressured = True
                            break
                if cand is None or (need <= 0 and not pressured):
                    return
                c, t = cand[2].popleft()
                t()
                need -= c

        for m16 in range(4):            # V blocks 0-3 (needed by span 0)
            for _, t in v_thunks(m16):
                t()

        acTs.extend(
            aTp.tile([P, GD // P, SPAN], bf, tag="acT", name=f"acT{J}")
            for J in range(NSPAN))
        for (J, hp) in PAIR_ORDER:
            attn_pair(J, hp, acTs[J], pull)

        for _, _, dq in sched:
            while dq:
                dq.popleft()[1]()
        drain_transposes(all_=True)
        for mo in range(C // P):
            for _, t in proj_thunks(3, mo):
                t()


def _emit(tc, mybir, reps=1):
    nc = tc.nc
    dt = mybir.dt
    f32, bf = dt.float32, dt.bfloat16

    xT_d = nc.dram_tensor("xT", [C, N], bf, kind="ExternalInput").ap()
    wqkT_d = nc.dram_tensor("wqkT", [C, 2 * GD], bf, kind="ExternalInput").ap()
    wvT_d = nc.dram_tensor("wvT", [C, GD], bf, kind="ExternalInput").ap()
    wpT_d = nc.dram_tensor("wpT", [GD, C], bf, kind="ExternalInput").ap()
    tri_d = nc.dram_tensor("TRI", [P, 2, P], bf, kind="ExternalInput").ap()
    id_d = nc.dram_tensor("I128", [P, P], bf, kind="ExternalInput").ap()
    out_d = nc.dram_tensor("outT", [C, N], bf, kind="ExternalOutput").ap()

    for _rep in range(reps):
        _emit_once(tc, mybir, xT_d, wqkT_d, wvT_d, wpT_d, tri_d, id_d, out_d)


def _get_module(reps=1):
    key = reps
    if key not in _CACHE:
        import concourse.tile as tile
        from concourse import bacc, mybir

        nc = bacc.Bacc("TRN2", target_bir_lowering=False, debug=False,
                       num_devices=8)
        with tile.TileContext(nc) as tc:
            _emit(tc, mybir, reps=reps)
        nc.compile()
        _CACHE[key] = nc
    return _CACHE[key]


def _host_inputs(x, w_qkv, w_proj):
    scale = D ** -0.5
    tri = np.zeros((P, 2, P), np.float32)
    for p in range(P):
        tri[p, :, p:] = 1.0
    tri = tri.astype(BF16)
    ident = np.eye(P, dtype=BF16)
    in_maps = []
    for core in range(8):
        b, g = core // 2, core % 2
        rows = slice(g * GD, (g + 1) * GD)
        wq = w_qkv[0 * C:1 * C][rows] * scale
        wk = w_qkv[1 * C:2 * C][rows]
        wv = w_qkv[2 * C:3 * C][rows]
        in_maps.append({
            "xT": np.ascontiguousarray(x[b].T).astype(BF16),
            "wqkT": np.ascontiguousarray(
                np.concatenate([wq, wk], axis=0).T).astype(BF16),
            "wvT": np.ascontiguousarray(wv.T).astype(BF16),
            "wpT": np.ascontiguousarray(w_proj[:, rows].T).astype(BF16),
            "TRI": tri,
            "I128": ident,
        })
    return in_maps


def kernel(x, w_qkv, w_proj, b_proj, _trace=False):
    from concourse.bass_utils import run_bass_kernel_spmd

    nc = _get_module()
    in_maps = _host_inputs(np.asarray(x, np.float32),
                           np.asarray(w_qkv, np.float32),
                           np.asarray(w_proj, np.float32))
    res = run_bass_kernel_spmd(nc, in_maps, core_ids=list(range(8)),
                               trace=_trace)
    outs = [r["outT"].astype(np.float32) for r in res.results]
    out = np.empty((B, N, C), np.float32)
    bp = np.asarray(b_proj, np.float32)[None, :]
    for b in range(B):
        out[b] = outs[2 * b].T + outs[2 * b + 1].T + bp
    if _trace:
        kernel._last_results = res
    return out
